# revision 23
# baseline (speedup 1.0000x reference)
"""Trainium2 Bass kernel for nn_Attention_60189671686666 (gnn_message_passing).

Math per batch b (N=128 atoms, H=256 feats):
    P        = X @ W_pair                       (N, H)   [pre-projection trick]
    S_raw[i,j] = sum_k relu(P[i,k] + P[j,k]) * W_att[k]  (symmetric in i,j)
    S        = sigmoid(S_raw + b_att)
    out      = S @ X

Distribution: data-parallel over batch B=32 across 8 NeuronCores (4 each).
No collectives needed; params are replicated.

On-core layout: P is kept transposed (P_T: feature k on partitions, atom n on
free axis) so the per-column bias P[:, i] is a per-partition scalar for
tensor_scalar / activation. The relu'd pairwise tile is consumed immediately
by a PE matmul against W_att, accumulating score columns in PSUM.
"""

import sys
import numpy as np

if "/opt/trn_rl_repo" not in sys.path:
    sys.path.insert(0, "/opt/trn_rl_repo")

B, N, H = 32, 128, 256
NCORES = 8
BL = B // NCORES  # batches per core

_CACHE = {}

# debug knobs (bisection): stages = 0 io-only, 1 +transpose/P, 2 +scores, 3 full
import os
DBG_STAGE = int(os.environ.get("K_DBG_STAGE", "3"))
DBG_NB = int(os.environ.get("K_DBG_NB", str(BL)))
DBG_NI = int(os.environ.get("K_DBG_NI", str(N)))


def _build_nc():
    import concourse.bass as bass
    import concourse.tile as tile
    from concourse import bacc, mybir

    f32 = mybir.dt.float32
    bf16 = mybir.dt.bfloat16
    RELU = mybir.ActivationFunctionType.Relu
    SIGMOID = mybir.ActivationFunctionType.Sigmoid
    ADD = mybir.AluOpType.add
    MAX = mybir.AluOpType.max

    nc = bacc.Bacc(None, target_bir_lowering=False, debug=False)
    lf = nc.declare_dram_parameter("local_feats", [BL, N, H], f32, isOutput=False)
    wp = nc.declare_dram_parameter("W_pair", [H, H], f32, isOutput=False)
    wa = nc.declare_dram_parameter("W_att", [H, 1], f32, isOutput=False)
    ba = nc.declare_dram_parameter("b_att", [1], f32, isOutput=False)
    out = nc.declare_dram_parameter("out", [BL, N, H], f32, isOutput=True)

    ident = nc.inline_tensor(np.eye(128, dtype=np.float32), name="ident")
    import ml_dtypes
    bf = ml_dtypes.bfloat16
    ident_bf = nc.inline_tensor(np.eye(128, dtype=bf), name="ident_bf")
    # s layout is [j (partitions), i (free)]; lower = j >= i
    mask_low = nc.inline_tensor(np.triu(np.ones((128, 128))).astype(bf),
                                name="mask_low")
    mask_strict = nc.inline_tensor(np.triu(np.ones((128, 128)), k=1).astype(bf),
                                   name="mask_strict")

    with tile.TileContext(nc) as tc:
        with (
            tc.tile_pool(name="const", bufs=1) as cpool,
            tc.tile_pool(name="work", bufs=2) as wpool,
            tc.tile_pool(name="atiles", bufs=16) as apool,
            tc.tile_pool(name="psum", bufs=2, space="PSUM") as ppool,
            tc.tile_pool(name="psumS", bufs=2, space="PSUM") as spool,
        ):
            # ---- constants (once per core) ----
            # batch-0 features first so the transpose/P pipeline starts early
            x0_sb = wpool.tile([128, H], f32, tag="x")
            nc.sync.dma_start(x0_sb[:], lf[0])

            id_sb = cpool.tile([128, 128], f32)
            nc.sync.dma_start(id_sb[:], ident[:])
            id_bf_sb = cpool.tile([128, 128], bf16)
            nc.sync.dma_start(id_bf_sb[:], ident_bf[:])
            mlow_sb = cpool.tile([128, 128], bf16)
            nc.sync.dma_start(mlow_sb[:], mask_low[:])
            mstr_sb = cpool.tile([128, 128], bf16)
            nc.sync.dma_start(mstr_sb[:], mask_strict[:])

            # W_pair rows h on partitions: wp_sb[:, ht*H + k] = W_pair[ht*128 + p, k]
            wp_sb = cpool.tile([128, 2 * H], f32)
            for ht in range(2):
                nc.sync.dma_start(wp_sb[:, ht * H:(ht + 1) * H],
                                  wp[ht * 128:(ht + 1) * 128, :])

            w_sb = cpool.tile([128, 2], f32)
            nc.sync.dma_start(w_sb[:].rearrange("p (t o) -> p t o", t=2),
                              wa.rearrange("(t p) o -> p t o", p=128))
            w_bf = cpool.tile([128, 2], bf16)
            nc.vector.tensor_copy(w_bf[:], w_sb[:])

            # broadcast b_att to all 128 partitions with a 0-stride DMA read
            bias_sb = cpool.tile([128, 1], f32)
            ba_bc = ba[None, :]
            ba_bc.ap[0] = [0, 128]
            nc.sync.dma_start(bias_sb[:], ba_bc)

            # ---- per-batch pipeline ----
            for ib in range(DBG_NB):
                if ib == 0:
                    x_sb = x0_sb
                else:
                    x_sb = wpool.tile([128, H], f32, tag="x")
                    nc.sync.dma_start(x_sb[:], lf[ib])

                if DBG_STAGE == 0:
                    o_sb = wpool.tile([128, H], f32, tag="osb")
                    nc.vector.tensor_copy(o_sb[:], x_sb[:])
                    nc.sync.dma_start(out[ib], o_sb[:])
                    continue

                # X_T: (h on partitions, n on free), 2 h-tiles via PE transpose
                xT_sb = wpool.tile([128, H], f32, tag="xT")
                for ht in range(2):
                    xT_ps = ppool.tile([128, 128], f32, tag="xTp")
                    nc.tensor.transpose(xT_ps[:], x_sb[:, ht * 128:(ht + 1) * 128],
                                        id_sb[:])
                    nc.vector.tensor_copy(xT_sb[:, ht * 128:(ht + 1) * 128], xT_ps[:])

                # P_T[k, n] = sum_h W_pair[h, k] * X[n, h]
                pT_ps = ppool.tile([128, H], f32, tag="pT")
                for kt in range(2):
                    for ht in range(2):
                        nc.tensor.matmul(
                            pT_ps[:, kt * 128:(kt + 1) * 128],
                            lhsT=wp_sb[:, ht * H + kt * 128: ht * H + kt * 128 + 128],
                            rhs=xT_sb[:, ht * 128:(ht + 1) * 128],
                            start=(ht == 0), stop=(ht == 1))

                pT_bf = wpool.tile([128, H], bf16, tag="pTbf")
                nc.scalar.copy(pT_bf[:], pT_ps[:])
                pT_f = wpool.tile([128, H], f32, tag="pTf")
                nc.vector.tensor_copy(pT_f[:], pT_ps[:])
                x_bf = wpool.tile([128, H], bf16, tag="xbf")
                nc.vector.tensor_copy(x_bf[:], x_sb[:])

                if DBG_STAGE == 1:
                    o_sb = wpool.tile([128, H], f32, tag="osb")
                    nc.vector.tensor_copy(o_sb[:], pT_f[:])
                    nc.sync.dma_start(out[ib], o_sb[:])
                    continue

                # scores, block-upper triangle only (S symmetric): for column i
                # we produce relu tiles and matmul-accumulate j < 32*(i//32)+32
                # (output partitions start at 0). The lower-left junk part
                # of S_ps holds junk; it is masked off after sigmoid.
                S_ps = spool.tile([128, 128], f32, tag="S")
                nc.vector.memset(S_ps[:], 0.0)  # keep junk region finite
                for i in range(DBG_NI):
                    j1 = min(32 * (i // 32) + 32, 128)
                    for kt in range(2):
                        u = (2 * i + kt) % 32
                        a_t = apool.tile([128, 128], bf16, tag=f"a{kt}")
                        src = pT_bf[:, kt * 128: kt * 128 + j1]
                        bias_ap = pT_f[:, kt * 128 + i: kt * 128 + i + 1]
                        if u < 12:
                            nc.scalar.activation(a_t[:, 0:j1], src, RELU,
                                                 bias=bias_ap, scale=1.0)
                        else:
                            nc.vector.tensor_scalar(a_t[:, 0:j1], src, bias_ap,
                                                    0.0, ADD, MAX)
                        nc.tensor.matmul(S_ps[0:j1, i:i + 1],
                                         lhsT=a_t[:, 0:j1],
                                         rhs=w_bf[:, kt:kt + 1],
                                         start=(kt == 0), stop=(kt == 1))

                s_sig = wpool.tile([128, 128], bf16, tag="ssig")
                nc.scalar.activation(s_sig[:, 0:DBG_NI], S_ps[:, 0:DBG_NI], SIGMOID,
                                     bias=bias_sb[:, 0:1], scale=1.0)

                # block-lower sigmoid -> full symmetric S via two masked copies
                # + PE transpose of the strict-lower part
                s_lowz = wpool.tile([128, 128], bf16, tag="slowz")
                nc.vector.tensor_mul(s_lowz[:], s_sig[:], mlow_sb[:])
                s_str = wpool.tile([128, 128], bf16, tag="sstr")
                nc.vector.tensor_mul(s_str[:], s_sig[:], mstr_sb[:])
                ts_ps = ppool.tile([128, 128], bf16, tag="xTp")
                nc.tensor.transpose(ts_ps[:], s_str[:], id_bf_sb[:])
                s_strT = wpool.tile([128, 128], bf16, tag="sstrT")
                nc.scalar.copy(s_strT[:], ts_ps[:])

                if DBG_STAGE == 2:
                    o_sb = wpool.tile([128, H], f32, tag="osb")
                    nc.vector.tensor_copy(o_sb[:, 0:128], s_sig[:])
                    nc.vector.tensor_copy(o_sb[:, 128:256], s_sig[:])
                    nc.sync.dma_start(out[ib], o_sb[:])
                    continue

                # out[i,h] = sum_j S[i,j] X[j,h]; S_sig = s_lowz + s_strT^T so
                # accumulate two matmuls (lhsT is (j, i) for both pieces)
                o_ps = ppool.tile([128, H], f32, tag="ops")
                nc.tensor.matmul(o_ps[:], lhsT=s_lowz[:], rhs=x_bf[:],
                                 start=True, stop=False)
                nc.tensor.matmul(o_ps[:], lhsT=s_strT[:], rhs=x_bf[:],
                                 start=False, stop=True)
                o_sb = wpool.tile([128, H], f32, tag="osb")
                nc.scalar.copy(o_sb[:], o_ps[:])
                nc.sync.dma_start(out[ib], o_sb[:])

    nc.compile()
    return nc


def _get_nc():
    if "nc" not in _CACHE:
        _CACHE["nc"] = _build_nc()
    return _CACHE["nc"]


def run(inputs, trace=False, **kw):
    from concourse.bass_utils import run_bass_kernel_spmd

    nc = _get_nc()
    lf = np.ascontiguousarray(inputs["local_feats"], dtype=np.float32)
    wp = np.ascontiguousarray(inputs["W_pair"], dtype=np.float32)
    wa = np.ascontiguousarray(inputs["W_att"], dtype=np.float32)
    ba = np.ascontiguousarray(inputs["b_att"], dtype=np.float32)
    in_maps = [
        {"local_feats": lf[c * BL:(c + 1) * BL], "W_pair": wp, "W_att": wa,
         "b_att": ba}
        for c in range(NCORES)
    ]
    res = run_bass_kernel_spmd(nc, in_maps, core_ids=list(range(NCORES)),
                               trace=trace, **kw)
    outp = np.concatenate([res.results[c]["out"] for c in range(NCORES)], axis=0)
    return outp.astype(np.float32), res


def kernel(**inputs):
    outp, _ = run(inputs, trace=False)
    return outp


# revision 24
# speedup vs baseline: 1.1980x; 1.1980x over previous
"""Trainium2 Bass kernel for nn_Attention_60189671686666 (gnn_message_passing).

Math per batch b (N=128 atoms, H=256 feats):
    P        = X @ W_pair                       (N, H)   [pre-projection trick]
    S_raw[i,j] = sum_k relu(P[i,k] + P[j,k]) * W_att[k]  (symmetric in i,j)
    S        = sigmoid(S_raw + b_att)
    out      = S @ X

Distribution: data-parallel over batch B=32 across 8 NeuronCores (4 each).
No collectives needed; params are replicated.

On-core layout: P is kept transposed (P_T: feature k on partitions, atom n on
free axis) so the per-column bias P[:, i] is a per-partition scalar for
tensor_scalar / activation. The relu'd pairwise tile is consumed immediately
by a PE matmul against W_att, accumulating score columns in PSUM.
"""

import sys
import numpy as np

if "/opt/trn_rl_repo" not in sys.path:
    sys.path.insert(0, "/opt/trn_rl_repo")

B, N, H = 32, 128, 256
NCORES = 8
BL = B // NCORES  # batches per core

_CACHE = {}

# debug knobs (bisection): stages = 0 io-only, 1 +transpose/P, 2 +scores, 3 full
import os
DBG_STAGE = int(os.environ.get("K_DBG_STAGE", "3"))
DBG_NB = int(os.environ.get("K_DBG_NB", str(BL)))
DBG_NI = int(os.environ.get("K_DBG_NI", str(N)))


def _build_nc():
    import concourse.bass as bass
    import concourse.tile as tile
    from concourse import bacc, mybir

    f32 = mybir.dt.float32
    bf16 = mybir.dt.bfloat16
    RELU = mybir.ActivationFunctionType.Relu
    SIGMOID = mybir.ActivationFunctionType.Sigmoid
    ADD = mybir.AluOpType.add
    MAX = mybir.AluOpType.max

    nc = bacc.Bacc(None, target_bir_lowering=False, debug=False)
    lf = nc.declare_dram_parameter("local_feats", [BL, N, H], f32, isOutput=False)
    wp = nc.declare_dram_parameter("W_pair", [H, H], f32, isOutput=False)
    wa = nc.declare_dram_parameter("W_att", [H, 1], f32, isOutput=False)
    ba = nc.declare_dram_parameter("b_att", [1], f32, isOutput=False)
    out = nc.declare_dram_parameter("out", [BL, N, H], f32, isOutput=True)

    ident = nc.inline_tensor(np.eye(128, dtype=np.float32), name="ident")

    with tile.TileContext(nc) as tc:
        with (
            tc.tile_pool(name="const", bufs=1) as cpool,
            tc.tile_pool(name="work", bufs=2) as wpool,
            tc.tile_pool(name="atiles", bufs=16) as apool,
            tc.tile_pool(name="psum", bufs=2, space="PSUM") as ppool,
            tc.tile_pool(name="psumS", bufs=2, space="PSUM") as spool,
        ):
            # ---- constants (once per core) ----
            # batch-0 features first so the transpose/P pipeline starts early
            x0_sb = wpool.tile([128, H], f32, tag="x")
            nc.sync.dma_start(x0_sb[:], lf[0])

            id_sb = cpool.tile([128, 128], f32)
            nc.sync.dma_start(id_sb[:], ident[:])

            # W_pair rows h on partitions: wp_sb[:, ht*H + k] = W_pair[ht*128 + p, k]
            wp_sb = cpool.tile([128, 2 * H], f32)
            for ht in range(2):
                nc.sync.dma_start(wp_sb[:, ht * H:(ht + 1) * H],
                                  wp[ht * 128:(ht + 1) * 128, :])

            w_sb = cpool.tile([128, 2], f32)
            nc.sync.dma_start(w_sb[:].rearrange("p (t o) -> p t o", t=2),
                              wa.rearrange("(t p) o -> p t o", p=128))
            w_bf = cpool.tile([128, 2], bf16)
            nc.vector.tensor_copy(w_bf[:], w_sb[:])

            # broadcast b_att to all 128 partitions with a 0-stride DMA read
            bias_sb = cpool.tile([128, 1], f32)
            ba_bc = ba[None, :]
            ba_bc.ap[0] = [0, 128]
            nc.sync.dma_start(bias_sb[:], ba_bc)

            # ---- per-batch pipeline ----
            for ib in range(DBG_NB):
                if ib == 0:
                    x_sb = x0_sb
                else:
                    x_sb = wpool.tile([128, H], f32, tag="x")
                    nc.sync.dma_start(x_sb[:], lf[ib])

                if DBG_STAGE == 0:
                    o_sb = wpool.tile([128, H], f32, tag="osb")
                    nc.vector.tensor_copy(o_sb[:], x_sb[:])
                    nc.sync.dma_start(out[ib], o_sb[:])
                    continue

                # X_T: (h on partitions, n on free), 2 h-tiles via PE transpose
                xT_sb = wpool.tile([128, H], f32, tag="xT")
                for ht in range(2):
                    xT_ps = ppool.tile([128, 128], f32, tag="xTp")
                    nc.tensor.transpose(xT_ps[:], x_sb[:, ht * 128:(ht + 1) * 128],
                                        id_sb[:])
                    nc.vector.tensor_copy(xT_sb[:, ht * 128:(ht + 1) * 128], xT_ps[:])

                # P_T[k, n] = sum_h W_pair[h, k] * X[n, h]
                pT_ps = ppool.tile([128, H], f32, tag="pT")
                for kt in range(2):
                    for ht in range(2):
                        nc.tensor.matmul(
                            pT_ps[:, kt * 128:(kt + 1) * 128],
                            lhsT=wp_sb[:, ht * H + kt * 128: ht * H + kt * 128 + 128],
                            rhs=xT_sb[:, ht * 128:(ht + 1) * 128],
                            start=(ht == 0), stop=(ht == 1))

                pT_bf = wpool.tile([128, H], bf16, tag="pTbf")
                nc.scalar.copy(pT_bf[:], pT_ps[:])
                pT_f = wpool.tile([128, H], f32, tag="pTf")
                nc.vector.tensor_copy(pT_f[:], pT_ps[:])
                x_bf = wpool.tile([128, H], bf16, tag="xbf")
                nc.vector.tensor_copy(x_bf[:], x_sb[:])

                if DBG_STAGE == 1:
                    o_sb = wpool.tile([128, H], f32, tag="osb")
                    nc.vector.tensor_copy(o_sb[:], pT_f[:])
                    nc.sync.dma_start(out[ib], o_sb[:])
                    continue

                # scores: S_ps[j, i] = sum_k relu(P[j,k] + P[i,k]) w[k]
                S_ps = spool.tile([128, 128], f32, tag="S")
                for i in range(DBG_NI):
                    for kt in range(2):
                        u = (2 * i + kt) % 32
                        a_t = apool.tile([128, 128], bf16, tag=f"a{kt}")
                        src = pT_bf[:, kt * 128:(kt + 1) * 128]
                        bias_ap = pT_f[:, kt * 128 + i: kt * 128 + i + 1]
                        if u < 11:
                            nc.scalar.activation(a_t[:], src, RELU,
                                                 bias=bias_ap, scale=1.0)
                        else:
                            nc.vector.tensor_scalar(a_t[:], src, bias_ap, 0.0,
                                                    ADD, MAX)
                        nc.tensor.matmul(S_ps[:, i:i + 1], lhsT=a_t[:],
                                         rhs=w_bf[:, kt:kt + 1],
                                         start=(kt == 0), stop=(kt == 1))

                s_sig = wpool.tile([128, 128], bf16, tag="ssig")
                nc.scalar.activation(s_sig[:, 0:DBG_NI], S_ps[:, 0:DBG_NI], SIGMOID,
                                     bias=bias_sb[:, 0:1], scale=1.0)

                if DBG_STAGE == 2:
                    o_sb = wpool.tile([128, H], f32, tag="osb")
                    nc.vector.tensor_copy(o_sb[:, 0:128], s_sig[:])
                    nc.vector.tensor_copy(o_sb[:, 128:256], s_sig[:])
                    nc.sync.dma_start(out[ib], o_sb[:])
                    continue

                # out[i, h] = sum_j S[i,j] X[j,h]; S symmetric so lhsT = S works
                o_ps = ppool.tile([128, H], f32, tag="ops")
                nc.tensor.matmul(o_ps[:], lhsT=s_sig[:], rhs=x_bf[:],
                                 start=True, stop=True)
                o_sb = wpool.tile([128, H], f32, tag="osb")
                nc.scalar.copy(o_sb[:], o_ps[:])
                nc.sync.dma_start(out[ib], o_sb[:])

    nc.compile()
    return nc


def _get_nc():
    if "nc" not in _CACHE:
        _CACHE["nc"] = _build_nc()
    return _CACHE["nc"]


def run(inputs, trace=False, **kw):
    from concourse.bass_utils import run_bass_kernel_spmd

    nc = _get_nc()
    lf = np.ascontiguousarray(inputs["local_feats"], dtype=np.float32)
    wp = np.ascontiguousarray(inputs["W_pair"], dtype=np.float32)
    wa = np.ascontiguousarray(inputs["W_att"], dtype=np.float32)
    ba = np.ascontiguousarray(inputs["b_att"], dtype=np.float32)
    in_maps = [
        {"local_feats": lf[c * BL:(c + 1) * BL], "W_pair": wp, "W_att": wa,
         "b_att": ba}
        for c in range(NCORES)
    ]
    res = run_bass_kernel_spmd(nc, in_maps, core_ids=list(range(NCORES)),
                               trace=trace, **kw)
    outp = np.concatenate([res.results[c]["out"] for c in range(NCORES)], axis=0)
    return outp.astype(np.float32), res


def kernel(**inputs):
    outp, _ = run(inputs, trace=False)
    return outp


# revision 25
# speedup vs baseline: 1.2064x; 1.0070x over previous
"""Trainium2 Bass kernel for nn_Attention_60189671686666 (gnn_message_passing).

Math per batch b (N=128 atoms, H=256 feats):
    P        = X @ W_pair                       (N, H)   [pre-projection trick]
    S_raw[i,j] = sum_k relu(P[i,k] + P[j,k]) * W_att[k]  (symmetric in i,j)
    S        = sigmoid(S_raw + b_att)
    out      = S @ X

Distribution: data-parallel over batch B=32 across 8 NeuronCores (4 each).
No collectives needed; params are replicated.

On-core layout: P is kept transposed (P_T: feature k on partitions, atom n on
free axis) so the per-column bias P[:, i] is a per-partition scalar for
tensor_scalar / activation. The relu'd pairwise tile is consumed immediately
by a PE matmul against W_att, accumulating score columns in PSUM.
"""

import sys
import numpy as np

if "/opt/trn_rl_repo" not in sys.path:
    sys.path.insert(0, "/opt/trn_rl_repo")

B, N, H = 32, 128, 256
NCORES = 8
BL = B // NCORES  # batches per core

_CACHE = {}

# debug knobs (bisection): stages = 0 io-only, 1 +transpose/P, 2 +scores, 3 full
import os
DBG_STAGE = int(os.environ.get("K_DBG_STAGE", "3"))
DBG_NB = int(os.environ.get("K_DBG_NB", str(BL)))
DBG_NI = int(os.environ.get("K_DBG_NI", str(N)))


def _build_nc():
    import concourse.bass as bass
    import concourse.tile as tile
    from concourse import bacc, mybir

    f32 = mybir.dt.float32
    bf16 = mybir.dt.bfloat16
    RELU = mybir.ActivationFunctionType.Relu
    SIGMOID = mybir.ActivationFunctionType.Sigmoid
    ADD = mybir.AluOpType.add
    MAX = mybir.AluOpType.max

    nc = bacc.Bacc(None, target_bir_lowering=False, debug=False)
    lf = nc.declare_dram_parameter("local_feats", [BL, N, H], f32, isOutput=False)
    wp = nc.declare_dram_parameter("W_pair", [H, H], f32, isOutput=False)
    wa = nc.declare_dram_parameter("W_att", [H, 1], f32, isOutput=False)
    ba = nc.declare_dram_parameter("b_att", [1], f32, isOutput=False)
    out = nc.declare_dram_parameter("out", [BL, N, H], f32, isOutput=True)

    ident = nc.inline_tensor(np.eye(128, dtype=np.float32), name="ident")

    with tile.TileContext(nc) as tc:
        with (
            tc.tile_pool(name="const", bufs=1) as cpool,
            tc.tile_pool(name="work", bufs=3) as wpool,
            tc.tile_pool(name="atiles", bufs=16) as apool,
            tc.tile_pool(name="psum", bufs=2, space="PSUM") as ppool,
            tc.tile_pool(name="psumS", bufs=2, space="PSUM") as spool,
        ):
            # ---- constants (once per core) ----
            # batch-0 features first so the transpose/P pipeline starts early
            x0_sb = wpool.tile([128, H], f32, tag="x")
            nc.sync.dma_start(x0_sb[:], lf[0])

            id_sb = cpool.tile([128, 128], f32)
            nc.sync.dma_start(id_sb[:], ident[:])

            # W_pair rows h on partitions: wp_sb[:, ht*H + k] = W_pair[ht*128 + p, k]
            wp_sb = cpool.tile([128, 2 * H], f32)
            for ht in range(2):
                nc.sync.dma_start(wp_sb[:, ht * H:(ht + 1) * H],
                                  wp[ht * 128:(ht + 1) * 128, :])

            w_sb = cpool.tile([128, 2], f32)
            nc.sync.dma_start(w_sb[:].rearrange("p (t o) -> p t o", t=2),
                              wa.rearrange("(t p) o -> p t o", p=128))
            w_bf = cpool.tile([128, 2], bf16)
            nc.vector.tensor_copy(w_bf[:], w_sb[:])

            # broadcast b_att to all 128 partitions with a 0-stride DMA read
            bias_sb = cpool.tile([128, 1], f32)
            ba_bc = ba[None, :]
            ba_bc.ap[0] = [0, 128]
            nc.sync.dma_start(bias_sb[:], ba_bc)

            # ---- per-batch pipeline ----
            for ib in range(DBG_NB):
                if ib == 0:
                    x_sb = x0_sb
                else:
                    x_sb = wpool.tile([128, H], f32, tag="x")
                    nc.sync.dma_start(x_sb[:], lf[ib])

                if DBG_STAGE == 0:
                    o_sb = wpool.tile([128, H], f32, tag="osb")
                    nc.vector.tensor_copy(o_sb[:], x_sb[:])
                    nc.sync.dma_start(out[ib], o_sb[:])
                    continue

                # X_T: (h on partitions, n on free), 2 h-tiles via PE transpose
                xT_sb = wpool.tile([128, H], f32, tag="xT")
                for ht in range(2):
                    xT_ps = ppool.tile([128, 128], f32, tag="xTp")
                    nc.tensor.transpose(xT_ps[:], x_sb[:, ht * 128:(ht + 1) * 128],
                                        id_sb[:])
                    nc.vector.tensor_copy(xT_sb[:, ht * 128:(ht + 1) * 128], xT_ps[:])

                # P_T[k, n] = sum_h W_pair[h, k] * X[n, h]
                pT_ps = ppool.tile([128, H], f32, tag="pT")
                for kt in range(2):
                    for ht in range(2):
                        nc.tensor.matmul(
                            pT_ps[:, kt * 128:(kt + 1) * 128],
                            lhsT=wp_sb[:, ht * H + kt * 128: ht * H + kt * 128 + 128],
                            rhs=xT_sb[:, ht * 128:(ht + 1) * 128],
                            start=(ht == 0), stop=(ht == 1))

                pT_bf = wpool.tile([128, H], bf16, tag="pTbf")
                nc.scalar.copy(pT_bf[:], pT_ps[:])
                pT_f = wpool.tile([128, H], f32, tag="pTf")
                nc.vector.tensor_copy(pT_f[:], pT_ps[:])
                x_bf = wpool.tile([128, H], bf16, tag="xbf")
                nc.vector.tensor_copy(x_bf[:], x_sb[:])

                if DBG_STAGE == 1:
                    o_sb = wpool.tile([128, H], f32, tag="osb")
                    nc.vector.tensor_copy(o_sb[:], pT_f[:])
                    nc.sync.dma_start(out[ib], o_sb[:])
                    continue

                # scores: S_ps[j, i] = sum_k relu(P[j,k] + P[i,k]) w[k]
                S_ps = spool.tile([128, 128], f32, tag="S")
                for i in range(DBG_NI):
                    for kt in range(2):
                        u = (2 * i + kt) % 32
                        a_t = apool.tile([128, 128], bf16, tag=f"a{kt}")
                        src = pT_bf[:, kt * 128:(kt + 1) * 128]
                        bias_ap = pT_f[:, kt * 128 + i: kt * 128 + i + 1]
                        if u < 11:
                            nc.scalar.activation(a_t[:], src, RELU,
                                                 bias=bias_ap, scale=1.0)
                        else:
                            nc.vector.tensor_scalar(a_t[:], src, bias_ap, 0.0,
                                                    ADD, MAX)
                        nc.tensor.matmul(S_ps[:, i:i + 1], lhsT=a_t[:],
                                         rhs=w_bf[:, kt:kt + 1],
                                         start=(kt == 0), stop=(kt == 1))

                s_sig = wpool.tile([128, 128], bf16, tag="ssig")
                nc.scalar.activation(s_sig[:, 0:DBG_NI], S_ps[:, 0:DBG_NI], SIGMOID,
                                     bias=bias_sb[:, 0:1], scale=1.0)

                if DBG_STAGE == 2:
                    o_sb = wpool.tile([128, H], f32, tag="osb")
                    nc.vector.tensor_copy(o_sb[:, 0:128], s_sig[:])
                    nc.vector.tensor_copy(o_sb[:, 128:256], s_sig[:])
                    nc.sync.dma_start(out[ib], o_sb[:])
                    continue

                # out[i, h] = sum_j S[i,j] X[j,h]; S symmetric so lhsT = S works
                o_ps = ppool.tile([128, H], f32, tag="ops")
                nc.tensor.matmul(o_ps[:], lhsT=s_sig[:], rhs=x_bf[:],
                                 start=True, stop=True)
                o_sb = wpool.tile([128, H], f32, tag="osb")
                nc.scalar.copy(o_sb[:], o_ps[:])
                nc.sync.dma_start(out[ib], o_sb[:])

    nc.compile()
    return nc


def _get_nc():
    if "nc" not in _CACHE:
        _CACHE["nc"] = _build_nc()
    return _CACHE["nc"]


def run(inputs, trace=False, **kw):
    from concourse.bass_utils import run_bass_kernel_spmd

    nc = _get_nc()
    lf = np.ascontiguousarray(inputs["local_feats"], dtype=np.float32)
    wp = np.ascontiguousarray(inputs["W_pair"], dtype=np.float32)
    wa = np.ascontiguousarray(inputs["W_att"], dtype=np.float32)
    ba = np.ascontiguousarray(inputs["b_att"], dtype=np.float32)
    in_maps = [
        {"local_feats": lf[c * BL:(c + 1) * BL], "W_pair": wp, "W_att": wa,
         "b_att": ba}
        for c in range(NCORES)
    ]
    res = run_bass_kernel_spmd(nc, in_maps, core_ids=list(range(NCORES)),
                               trace=trace, **kw)
    outp = np.concatenate([res.results[c]["out"] for c in range(NCORES)], axis=0)
    return outp.astype(np.float32), res


def kernel(**inputs):
    outp, _ = run(inputs, trace=False)
    return outp


# revision 26
# speedup vs baseline: 1.2101x; 1.0030x over previous
"""Trainium2 Bass kernel for nn_Attention_60189671686666 (gnn_message_passing).

Math per batch b (N=128 atoms, H=256 feats):
    P        = X @ W_pair                       (N, H)   [pre-projection trick]
    S_raw[i,j] = sum_k relu(P[i,k] + P[j,k]) * W_att[k]  (symmetric in i,j)
    S        = sigmoid(S_raw + b_att)
    out      = S @ X

Distribution: data-parallel over batch B=32 across 8 NeuronCores (4 each).
No collectives needed; params are replicated.

On-core layout: P is kept transposed (P_T: feature k on partitions, atom n on
free axis) so the per-column bias P[:, i] is a per-partition scalar for
tensor_scalar / activation. The relu'd pairwise tile (bf16) is consumed
immediately by a PE matmul against W_att, accumulating score columns in PSUM.
Produce work is split DVE:ACT ~66:34 (per-tile cost ~163ns vs ~293ns, both
dominated by the per-instruction SBUF-access bubble, which bounds this kernel).
Measured ~139 us on hardware, rel err 2.6e-3.
"""

import sys
import numpy as np

if "/opt/trn_rl_repo" not in sys.path:
    sys.path.insert(0, "/opt/trn_rl_repo")

B, N, H = 32, 128, 256
NCORES = 8
BL = B // NCORES  # batches per core

_CACHE = {}

# debug knobs (bisection): stages = 0 io-only, 1 +transpose/P, 2 +scores, 3 full
import os
DBG_STAGE = int(os.environ.get("K_DBG_STAGE", "3"))
DBG_NB = int(os.environ.get("K_DBG_NB", str(BL)))
DBG_NI = int(os.environ.get("K_DBG_NI", str(N)))


def _build_nc():
    import concourse.bass as bass
    import concourse.tile as tile
    from concourse import bacc, mybir

    f32 = mybir.dt.float32
    bf16 = mybir.dt.bfloat16
    RELU = mybir.ActivationFunctionType.Relu
    SIGMOID = mybir.ActivationFunctionType.Sigmoid
    ADD = mybir.AluOpType.add
    MAX = mybir.AluOpType.max

    nc = bacc.Bacc(None, target_bir_lowering=False, debug=False)
    lf = nc.declare_dram_parameter("local_feats", [BL, N, H], f32, isOutput=False)
    wp = nc.declare_dram_parameter("W_pair", [H, H], f32, isOutput=False)
    wa = nc.declare_dram_parameter("W_att", [H, 1], f32, isOutput=False)
    ba = nc.declare_dram_parameter("b_att", [1], f32, isOutput=False)
    out = nc.declare_dram_parameter("out", [BL, N, H], f32, isOutput=True)

    ident = nc.inline_tensor(np.eye(128, dtype=np.float32), name="ident")

    with tile.TileContext(nc) as tc:
        with (
            tc.tile_pool(name="const", bufs=1) as cpool,
            tc.tile_pool(name="work", bufs=3) as wpool,
            tc.tile_pool(name="atiles", bufs=16) as apool,
            tc.tile_pool(name="psum", bufs=2, space="PSUM") as ppool,
            tc.tile_pool(name="psumS", bufs=2, space="PSUM") as spool,
        ):
            # ---- constants (once per core) ----
            # batch-0 features first so the transpose/P pipeline starts early
            x0_sb = wpool.tile([128, H], f32, tag="x")
            nc.sync.dma_start(x0_sb[:], lf[0])

            id_sb = cpool.tile([128, 128], f32)
            nc.sync.dma_start(id_sb[:], ident[:])

            # W_pair rows h on partitions: wp_sb[:, ht*H + k] = W_pair[ht*128 + p, k]
            wp_sb = cpool.tile([128, 2 * H], f32)
            for ht in range(2):
                nc.sync.dma_start(wp_sb[:, ht * H:(ht + 1) * H],
                                  wp[ht * 128:(ht + 1) * 128, :])

            w_sb = cpool.tile([128, 2], f32)
            nc.sync.dma_start(w_sb[:].rearrange("p (t o) -> p t o", t=2),
                              wa.rearrange("(t p) o -> p t o", p=128))
            w_bf = cpool.tile([128, 2], bf16)
            nc.vector.tensor_copy(w_bf[:], w_sb[:])

            # broadcast b_att to all 128 partitions with a 0-stride DMA read
            bias_sb = cpool.tile([128, 1], f32)
            ba_bc = ba[None, :]
            ba_bc.ap[0] = [0, 128]
            nc.sync.dma_start(bias_sb[:], ba_bc)

            # ---- per-batch pipeline ----
            for ib in range(DBG_NB):
                if ib == 0:
                    x_sb = x0_sb
                else:
                    x_sb = wpool.tile([128, H], f32, tag="x")
                    nc.sync.dma_start(x_sb[:], lf[ib])

                if DBG_STAGE == 0:
                    o_sb = wpool.tile([128, H], f32, tag="osb")
                    nc.vector.tensor_copy(o_sb[:], x_sb[:])
                    nc.sync.dma_start(out[ib], o_sb[:])
                    continue

                # X_T: (h on partitions, n on free), 2 h-tiles via PE transpose
                xT_sb = wpool.tile([128, H], f32, tag="xT")
                for ht in range(2):
                    xT_ps = ppool.tile([128, 128], f32, tag="xTp")
                    nc.tensor.transpose(xT_ps[:], x_sb[:, ht * 128:(ht + 1) * 128],
                                        id_sb[:])
                    nc.vector.tensor_copy(xT_sb[:, ht * 128:(ht + 1) * 128], xT_ps[:])

                # P_T[k, n] = sum_h W_pair[h, k] * X[n, h]
                pT_ps = ppool.tile([128, H], f32, tag="pT")
                for kt in range(2):
                    for ht in range(2):
                        nc.tensor.matmul(
                            pT_ps[:, kt * 128:(kt + 1) * 128],
                            lhsT=wp_sb[:, ht * H + kt * 128: ht * H + kt * 128 + 128],
                            rhs=xT_sb[:, ht * 128:(ht + 1) * 128],
                            start=(ht == 0), stop=(ht == 1))

                pT_bf = wpool.tile([128, H], bf16, tag="pTbf")
                nc.scalar.copy(pT_bf[:], pT_ps[:])
                pT_f = wpool.tile([128, H], f32, tag="pTf")
                nc.vector.tensor_copy(pT_f[:], pT_ps[:])
                x_bf = wpool.tile([128, H], bf16, tag="xbf")
                nc.vector.tensor_copy(x_bf[:], x_sb[:])

                if DBG_STAGE == 1:
                    o_sb = wpool.tile([128, H], f32, tag="osb")
                    nc.vector.tensor_copy(o_sb[:], pT_f[:])
                    nc.sync.dma_start(out[ib], o_sb[:])
                    continue

                # scores: S_ps[j, i] = sum_k relu(P[j,k] + P[i,k]) w[k]
                S_ps = spool.tile([128, 128], f32, tag="S")
                for i in range(DBG_NI):
                    for kt in range(2):
                        u = (2 * i + kt) % 32
                        a_t = apool.tile([128, 128], bf16, tag=f"a{kt}")
                        src = pT_bf[:, kt * 128:(kt + 1) * 128]
                        bias_ap = pT_f[:, kt * 128 + i: kt * 128 + i + 1]
                        if u < 11:
                            nc.scalar.activation(a_t[:], src, RELU,
                                                 bias=bias_ap, scale=1.0)
                        else:
                            nc.vector.tensor_scalar(a_t[:], src, bias_ap, 0.0,
                                                    ADD, MAX)
                        nc.tensor.matmul(S_ps[:, i:i + 1], lhsT=a_t[:],
                                         rhs=w_bf[:, kt:kt + 1],
                                         start=(kt == 0), stop=(kt == 1))

                s_sig = wpool.tile([128, 128], bf16, tag="ssig")
                nc.scalar.activation(s_sig[:, 0:DBG_NI], S_ps[:, 0:DBG_NI], SIGMOID,
                                     bias=bias_sb[:, 0:1], scale=1.0)

                if DBG_STAGE == 2:
                    o_sb = wpool.tile([128, H], f32, tag="osb")
                    nc.vector.tensor_copy(o_sb[:, 0:128], s_sig[:])
                    nc.vector.tensor_copy(o_sb[:, 128:256], s_sig[:])
                    nc.sync.dma_start(out[ib], o_sb[:])
                    continue

                # out[i, h] = sum_j S[i,j] X[j,h]; S symmetric so lhsT = S works
                o_ps = ppool.tile([128, H], f32, tag="ops")
                nc.tensor.matmul(o_ps[:], lhsT=s_sig[:], rhs=x_bf[:],
                                 start=True, stop=True)
                o_sb = wpool.tile([128, H], f32, tag="osb")
                nc.scalar.copy(o_sb[:], o_ps[:])
                nc.sync.dma_start(out[ib], o_sb[:])

    nc.compile()
    return nc


def _get_nc():
    if "nc" not in _CACHE:
        _CACHE["nc"] = _build_nc()
    return _CACHE["nc"]


def run(inputs, trace=False, **kw):
    from concourse.bass_utils import run_bass_kernel_spmd

    nc = _get_nc()
    lf = np.ascontiguousarray(inputs["local_feats"], dtype=np.float32)
    wp = np.ascontiguousarray(inputs["W_pair"], dtype=np.float32)
    wa = np.ascontiguousarray(inputs["W_att"], dtype=np.float32)
    ba = np.ascontiguousarray(inputs["b_att"], dtype=np.float32)
    in_maps = [
        {"local_feats": lf[c * BL:(c + 1) * BL], "W_pair": wp, "W_att": wa,
         "b_att": ba}
        for c in range(NCORES)
    ]
    res = run_bass_kernel_spmd(nc, in_maps, core_ids=list(range(NCORES)),
                               trace=trace, **kw)
    outp = np.concatenate([res.results[c]["out"] for c in range(NCORES)], axis=0)
    return outp.astype(np.float32), res


def kernel(**inputs):
    outp, _ = run(inputs, trace=False)
    return outp


# revision 28
# speedup vs baseline: 1.2115x; 1.0012x over previous
"""Trainium2 Bass kernel for nn_Attention_60189671686666 (gnn_message_passing).

Math per batch b (N=128 atoms, H=256 feats):
    P        = X @ W_pair                       (N, H)   [pre-projection trick]
    S_raw[i,j] = sum_k relu(P[i,k] + P[j,k]) * W_att[k]  (symmetric in i,j)
    S        = sigmoid(S_raw + b_att)
    out      = S @ X

Distribution: data-parallel over batch B=32 across 8 NeuronCores (4 each).
No collectives needed; params are replicated.

On-core layout: P is kept transposed (P_T: feature k on partitions, atom n on
free axis) so the per-column bias P[:, i] is a per-partition scalar for
tensor_scalar / activation. The relu'd pairwise tile (bf16) is consumed
immediately by a PE matmul against W_att, accumulating score columns in PSUM.
Produce work is split DVE:ACT ~66:34 (per-tile cost ~163ns vs ~293ns, both
dominated by the per-instruction SBUF-access bubble, which bounds this kernel).
Measured ~139 us on hardware, rel err 2.6e-3.
"""

import sys
import numpy as np

if "/opt/trn_rl_repo" not in sys.path:
    sys.path.insert(0, "/opt/trn_rl_repo")

B, N, H = 32, 128, 256
NCORES = 8
BL = B // NCORES  # batches per core

_CACHE = {}

# debug knobs (bisection): stages = 0 io-only, 1 +transpose/P, 2 +scores, 3 full
import os
DBG_STAGE = int(os.environ.get("K_DBG_STAGE", "3"))
DBG_NB = int(os.environ.get("K_DBG_NB", str(BL)))
DBG_NI = int(os.environ.get("K_DBG_NI", str(N)))


def _build_nc():
    import concourse.bass as bass
    import concourse.tile as tile
    from concourse import bacc, mybir

    f32 = mybir.dt.float32
    bf16 = mybir.dt.bfloat16
    RELU = mybir.ActivationFunctionType.Relu
    SIGMOID = mybir.ActivationFunctionType.Sigmoid
    ADD = mybir.AluOpType.add
    MAX = mybir.AluOpType.max

    nc = bacc.Bacc(None, target_bir_lowering=False, debug=False)
    lf = nc.declare_dram_parameter("local_feats", [BL, N, H], f32, isOutput=False)
    wp = nc.declare_dram_parameter("W_pair", [H, H], f32, isOutput=False)
    wa = nc.declare_dram_parameter("W_att", [H, 1], f32, isOutput=False)
    ba = nc.declare_dram_parameter("b_att", [1], f32, isOutput=False)
    out = nc.declare_dram_parameter("out", [BL, N, H], f32, isOutput=True)

    ident = nc.inline_tensor(np.eye(128, dtype=np.float32), name="ident")

    with tile.TileContext(nc) as tc:
        with (
            tc.tile_pool(name="const", bufs=1) as cpool,
            tc.tile_pool(name="work", bufs=3) as wpool,
            tc.tile_pool(name="atiles", bufs=16) as apool,
            tc.tile_pool(name="psum", bufs=2, space="PSUM") as ppool,
            tc.tile_pool(name="psumS", bufs=3, space="PSUM") as spool,
            tc.tile_pool(name="psumO", bufs=1, space="PSUM") as opool,
        ):
            # ---- constants (once per core) ----
            # batch-0 features first so the transpose/P pipeline starts early
            x0_sb = wpool.tile([128, H], f32, tag="x")
            nc.sync.dma_start(x0_sb[:], lf[0])

            id_sb = cpool.tile([128, 128], f32)
            nc.sync.dma_start(id_sb[:], ident[:])

            # W_pair rows h on partitions: wp_sb[:, ht*H + k] = W_pair[ht*128 + p, k]
            wp_sb = cpool.tile([128, 2 * H], f32)
            for ht in range(2):
                nc.sync.dma_start(wp_sb[:, ht * H:(ht + 1) * H],
                                  wp[ht * 128:(ht + 1) * 128, :])

            w_sb = cpool.tile([128, 2], f32)
            nc.sync.dma_start(w_sb[:].rearrange("p (t o) -> p t o", t=2),
                              wa.rearrange("(t p) o -> p t o", p=128))
            w_bf = cpool.tile([128, 2], bf16)
            nc.vector.tensor_copy(w_bf[:], w_sb[:])

            # broadcast b_att to all 128 partitions with a 0-stride DMA read
            bias_sb = cpool.tile([128, 1], f32)
            ba_bc = ba[None, :]
            ba_bc.ap[0] = [0, 128]
            nc.sync.dma_start(bias_sb[:], ba_bc)

            # ---- per-batch pipeline ----
            for ib in range(DBG_NB):
                if ib == 0:
                    x_sb = x0_sb
                else:
                    x_sb = wpool.tile([128, H], f32, tag="x")
                    nc.sync.dma_start(x_sb[:], lf[ib])

                if DBG_STAGE == 0:
                    o_sb = wpool.tile([128, H], f32, tag="osb")
                    nc.vector.tensor_copy(o_sb[:], x_sb[:])
                    nc.sync.dma_start(out[ib], o_sb[:])
                    continue

                # X_T: (h on partitions, n on free), 2 h-tiles via PE transpose
                xT_sb = wpool.tile([128, H], f32, tag="xT")
                for ht in range(2):
                    xT_ps = ppool.tile([128, 128], f32, tag="xTp")
                    nc.tensor.transpose(xT_ps[:], x_sb[:, ht * 128:(ht + 1) * 128],
                                        id_sb[:])
                    nc.vector.tensor_copy(xT_sb[:, ht * 128:(ht + 1) * 128], xT_ps[:])

                # P_T[k, n] = sum_h W_pair[h, k] * X[n, h]
                pT_ps = ppool.tile([128, H], f32, tag="pT")
                for kt in range(2):
                    for ht in range(2):
                        nc.tensor.matmul(
                            pT_ps[:, kt * 128:(kt + 1) * 128],
                            lhsT=wp_sb[:, ht * H + kt * 128: ht * H + kt * 128 + 128],
                            rhs=xT_sb[:, ht * 128:(ht + 1) * 128],
                            start=(ht == 0), stop=(ht == 1))

                # per-kt copies so kt0 produce can start while kt1's P matmuls
                # are still in flight
                pT_bf = wpool.tile([128, H], bf16, tag="pTbf")
                pT_f = wpool.tile([128, H], f32, tag="pTf")
                for kt in range(2):
                    sl = slice(kt * 128, (kt + 1) * 128)
                    nc.scalar.copy(pT_bf[:, sl], pT_ps[:, sl])
                    nc.vector.tensor_copy(pT_f[:, sl], pT_ps[:, sl])
                x_bf = wpool.tile([128, H], bf16, tag="xbf")
                nc.vector.tensor_copy(x_bf[:], x_sb[:])

                if DBG_STAGE == 1:
                    o_sb = wpool.tile([128, H], f32, tag="osb")
                    nc.vector.tensor_copy(o_sb[:], pT_f[:])
                    nc.sync.dma_start(out[ib], o_sb[:])
                    continue

                # scores: S_ps[j, i] = sum_k relu(P[j,k] + P[i,k]) w[k]
                S_ps = spool.tile([128, 128], f32, tag="S")
                for i in range(DBG_NI):
                    for kt in range(2):
                        u = (2 * i + kt) % 32
                        a_t = apool.tile([128, 128], bf16, tag=f"a{kt}")
                        src = pT_bf[:, kt * 128:(kt + 1) * 128]
                        bias_ap = pT_f[:, kt * 128 + i: kt * 128 + i + 1]
                        if u < 11:
                            nc.scalar.activation(a_t[:], src, RELU,
                                                 bias=bias_ap, scale=1.0)
                        else:
                            nc.vector.tensor_scalar(a_t[:], src, bias_ap, 0.0,
                                                    ADD, MAX)
                        nc.tensor.matmul(S_ps[:, i:i + 1], lhsT=a_t[:],
                                         rhs=w_bf[:, kt:kt + 1],
                                         start=(kt == 0), stop=(kt == 1))

                s_sig = wpool.tile([128, 128], bf16, tag="ssig")
                nc.scalar.activation(s_sig[:, 0:DBG_NI], S_ps[:, 0:DBG_NI], SIGMOID,
                                     bias=bias_sb[:, 0:1], scale=1.0)

                if DBG_STAGE == 2:
                    o_sb = wpool.tile([128, H], f32, tag="osb")
                    nc.vector.tensor_copy(o_sb[:, 0:128], s_sig[:])
                    nc.vector.tensor_copy(o_sb[:, 128:256], s_sig[:])
                    nc.sync.dma_start(out[ib], o_sb[:])
                    continue

                # out[i, h] = sum_j S[i,j] X[j,h]; S symmetric so lhsT = S works
                o_ps = opool.tile([128, H], f32, tag="ops")
                nc.tensor.matmul(o_ps[:], lhsT=s_sig[:], rhs=x_bf[:],
                                 start=True, stop=True)
                o_sb = wpool.tile([128, H], f32, tag="osb")
                nc.scalar.copy(o_sb[:], o_ps[:])
                nc.sync.dma_start(out[ib], o_sb[:])

    nc.compile()
    return nc


def _get_nc():
    if "nc" not in _CACHE:
        _CACHE["nc"] = _build_nc()
    return _CACHE["nc"]


def run(inputs, trace=False, **kw):
    from concourse.bass_utils import run_bass_kernel_spmd

    nc = _get_nc()
    lf = np.ascontiguousarray(inputs["local_feats"], dtype=np.float32)
    wp = np.ascontiguousarray(inputs["W_pair"], dtype=np.float32)
    wa = np.ascontiguousarray(inputs["W_att"], dtype=np.float32)
    ba = np.ascontiguousarray(inputs["b_att"], dtype=np.float32)
    in_maps = [
        {"local_feats": lf[c * BL:(c + 1) * BL], "W_pair": wp, "W_att": wa,
         "b_att": ba}
        for c in range(NCORES)
    ]
    res = run_bass_kernel_spmd(nc, in_maps, core_ids=list(range(NCORES)),
                               trace=trace, **kw)
    outp = np.concatenate([res.results[c]["out"] for c in range(NCORES)], axis=0)
    return outp.astype(np.float32), res


def kernel(**inputs):
    outp, _ = run(inputs, trace=False)
    return outp


# revision 29
# speedup vs baseline: 1.2138x; 1.0019x over previous
"""Trainium2 Bass kernel for nn_Attention_60189671686666 (gnn_message_passing).

Math per batch b (N=128 atoms, H=256 feats):
    P        = X @ W_pair                       (N, H)   [pre-projection trick]
    S_raw[i,j] = sum_k relu(P[i,k] + P[j,k]) * W_att[k]  (symmetric in i,j)
    S        = sigmoid(S_raw + b_att)
    out      = S @ X

Distribution: data-parallel over batch B=32 across 8 NeuronCores (4 each).
No collectives needed; params are replicated.

On-core layout: P is kept transposed (P_T: feature k on partitions, atom n on
free axis) so the per-column bias P[:, i] is a per-partition scalar for
tensor_scalar / activation. The relu'd pairwise tile (bf16) is consumed
immediately by a PE matmul against W_att, accumulating score columns in PSUM.
Produce work is split DVE:ACT ~66:34 (per-tile cost ~163ns vs ~293ns, both
dominated by the per-instruction SBUF-access bubble, which bounds this kernel:
8192 produce instructions fleet-wide is the floor for this ISA since the
per-partition bias pins each instruction to one 128x128 tile).
Measured ~137.5 us on hardware (DVE 83% busy), rel err 2.6e-3.
"""

import sys
import numpy as np

if "/opt/trn_rl_repo" not in sys.path:
    sys.path.insert(0, "/opt/trn_rl_repo")

B, N, H = 32, 128, 256
NCORES = 8
BL = B // NCORES  # batches per core

_CACHE = {}

# debug knobs (bisection): stages = 0 io-only, 1 +transpose/P, 2 +scores, 3 full
import os
DBG_STAGE = int(os.environ.get("K_DBG_STAGE", "3"))
DBG_NB = int(os.environ.get("K_DBG_NB", str(BL)))
DBG_NI = int(os.environ.get("K_DBG_NI", str(N)))


def _build_nc():
    import concourse.bass as bass
    import concourse.tile as tile
    from concourse import bacc, mybir

    f32 = mybir.dt.float32
    bf16 = mybir.dt.bfloat16
    RELU = mybir.ActivationFunctionType.Relu
    SIGMOID = mybir.ActivationFunctionType.Sigmoid
    ADD = mybir.AluOpType.add
    MAX = mybir.AluOpType.max

    nc = bacc.Bacc(None, target_bir_lowering=False, debug=False)
    lf = nc.declare_dram_parameter("local_feats", [BL, N, H], f32, isOutput=False)
    wp = nc.declare_dram_parameter("W_pair", [H, H], f32, isOutput=False)
    wa = nc.declare_dram_parameter("W_att", [H, 1], f32, isOutput=False)
    ba = nc.declare_dram_parameter("b_att", [1], f32, isOutput=False)
    out = nc.declare_dram_parameter("out", [BL, N, H], f32, isOutput=True)

    ident = nc.inline_tensor(np.eye(128, dtype=np.float32), name="ident")

    with tile.TileContext(nc) as tc:
        with (
            tc.tile_pool(name="const", bufs=1) as cpool,
            tc.tile_pool(name="work", bufs=3) as wpool,
            tc.tile_pool(name="atiles", bufs=16) as apool,
            tc.tile_pool(name="psum", bufs=2, space="PSUM") as ppool,
            tc.tile_pool(name="psumS", bufs=3, space="PSUM") as spool,
            tc.tile_pool(name="psumO", bufs=1, space="PSUM") as opool,
        ):
            # ---- constants (once per core) ----
            # batch-0 features first so the transpose/P pipeline starts early
            x0_sb = wpool.tile([128, H], f32, tag="x")
            nc.sync.dma_start(x0_sb[:], lf[0])

            id_sb = cpool.tile([128, 128], f32)
            nc.sync.dma_start(id_sb[:], ident[:])

            # W_pair rows h on partitions: wp_sb[:, ht*H + k] = W_pair[ht*128 + p, k]
            wp_sb = cpool.tile([128, 2 * H], f32)
            for ht in range(2):
                nc.sync.dma_start(wp_sb[:, ht * H:(ht + 1) * H],
                                  wp[ht * 128:(ht + 1) * 128, :])

            w_sb = cpool.tile([128, 2], f32)
            nc.sync.dma_start(w_sb[:].rearrange("p (t o) -> p t o", t=2),
                              wa.rearrange("(t p) o -> p t o", p=128))
            w_bf = cpool.tile([128, 2], bf16)
            nc.vector.tensor_copy(w_bf[:], w_sb[:])

            # broadcast b_att to all 128 partitions with a 0-stride DMA read
            bias_sb = cpool.tile([128, 1], f32)
            ba_bc = ba[None, :]
            ba_bc.ap[0] = [0, 128]
            nc.sync.dma_start(bias_sb[:], ba_bc)

            # ---- per-batch pipeline ----
            for ib in range(DBG_NB):
                if ib == 0:
                    x_sb = x0_sb
                else:
                    x_sb = wpool.tile([128, H], f32, tag="x")
                    nc.sync.dma_start(x_sb[:], lf[ib])

                if DBG_STAGE == 0:
                    o_sb = wpool.tile([128, H], f32, tag="osb")
                    nc.vector.tensor_copy(o_sb[:], x_sb[:])
                    nc.sync.dma_start(out[ib], o_sb[:])
                    continue

                # X_T: (h on partitions, n on free), 2 h-tiles via PE transpose
                xT_sb = wpool.tile([128, H], f32, tag="xT")
                for ht in range(2):
                    xT_ps = ppool.tile([128, 128], f32, tag="xTp")
                    nc.tensor.transpose(xT_ps[:], x_sb[:, ht * 128:(ht + 1) * 128],
                                        id_sb[:])
                    nc.vector.tensor_copy(xT_sb[:, ht * 128:(ht + 1) * 128], xT_ps[:])

                # P_T[k, n] = sum_h W_pair[h, k] * X[n, h]
                pT_ps = ppool.tile([128, H], f32, tag="pT")
                for kt in range(2):
                    for ht in range(2):
                        nc.tensor.matmul(
                            pT_ps[:, kt * 128:(kt + 1) * 128],
                            lhsT=wp_sb[:, ht * H + kt * 128: ht * H + kt * 128 + 128],
                            rhs=xT_sb[:, ht * 128:(ht + 1) * 128],
                            start=(ht == 0), stop=(ht == 1))

                # per-kt copies so kt0 produce can start while kt1's P matmuls
                # are still in flight
                pT_bf = wpool.tile([128, H], bf16, tag="pTbf")
                pT_f = wpool.tile([128, H], f32, tag="pTf")
                for kt in range(2):
                    sl = slice(kt * 128, (kt + 1) * 128)
                    nc.scalar.copy(pT_bf[:, sl], pT_ps[:, sl])
                    nc.vector.tensor_copy(pT_f[:, sl], pT_ps[:, sl])
                x_bf = wpool.tile([128, H], bf16, tag="xbf")
                nc.vector.tensor_copy(x_bf[:], x_sb[:])

                if DBG_STAGE == 1:
                    o_sb = wpool.tile([128, H], f32, tag="osb")
                    nc.vector.tensor_copy(o_sb[:], pT_f[:])
                    nc.sync.dma_start(out[ib], o_sb[:])
                    continue

                # scores: S_ps[j, i] = sum_k relu(P[j,k] + P[i,k]) w[k]
                S_ps = spool.tile([128, 128], f32, tag="S")
                for i in range(DBG_NI):
                    for kt in range(2):
                        u = (2 * i + kt) % 32
                        a_t = apool.tile([128, 128], bf16, tag=f"a{kt}")
                        src = pT_bf[:, kt * 128:(kt + 1) * 128]
                        bias_ap = pT_f[:, kt * 128 + i: kt * 128 + i + 1]
                        if u < 11:
                            nc.scalar.activation(a_t[:], src, RELU,
                                                 bias=bias_ap, scale=1.0)
                        else:
                            nc.vector.tensor_scalar(a_t[:], src, bias_ap, 0.0,
                                                    ADD, MAX)
                        nc.tensor.matmul(S_ps[:, i:i + 1], lhsT=a_t[:],
                                         rhs=w_bf[:, kt:kt + 1],
                                         start=(kt == 0), stop=(kt == 1))

                s_sig = wpool.tile([128, 128], bf16, tag="ssig")
                nc.scalar.activation(s_sig[:, 0:DBG_NI], S_ps[:, 0:DBG_NI], SIGMOID,
                                     bias=bias_sb[:, 0:1], scale=1.0)

                if DBG_STAGE == 2:
                    o_sb = wpool.tile([128, H], f32, tag="osb")
                    nc.vector.tensor_copy(o_sb[:, 0:128], s_sig[:])
                    nc.vector.tensor_copy(o_sb[:, 128:256], s_sig[:])
                    nc.sync.dma_start(out[ib], o_sb[:])
                    continue

                # out[i, h] = sum_j S[i,j] X[j,h]; S symmetric so lhsT = S works
                o_ps = opool.tile([128, H], f32, tag="ops")
                nc.tensor.matmul(o_ps[:], lhsT=s_sig[:], rhs=x_bf[:],
                                 start=True, stop=True)
                o_sb = wpool.tile([128, H], f32, tag="osb")
                nc.scalar.copy(o_sb[:], o_ps[:])
                nc.sync.dma_start(out[ib], o_sb[:])

    nc.compile()
    return nc


def _get_nc():
    if "nc" not in _CACHE:
        _CACHE["nc"] = _build_nc()
    return _CACHE["nc"]


def run(inputs, trace=False, **kw):
    from concourse.bass_utils import run_bass_kernel_spmd

    nc = _get_nc()
    lf = np.ascontiguousarray(inputs["local_feats"], dtype=np.float32)
    wp = np.ascontiguousarray(inputs["W_pair"], dtype=np.float32)
    wa = np.ascontiguousarray(inputs["W_att"], dtype=np.float32)
    ba = np.ascontiguousarray(inputs["b_att"], dtype=np.float32)
    in_maps = [
        {"local_feats": lf[c * BL:(c + 1) * BL], "W_pair": wp, "W_att": wa,
         "b_att": ba}
        for c in range(NCORES)
    ]
    res = run_bass_kernel_spmd(nc, in_maps, core_ids=list(range(NCORES)),
                               trace=trace, **kw)
    outp = np.concatenate([res.results[c]["out"] for c in range(NCORES)], axis=0)
    return outp.astype(np.float32), res


def kernel(**inputs):
    outp, _ = run(inputs, trace=False)
    return outp


# revision 30
# speedup vs baseline: 1.2170x; 1.0027x over previous
"""Trainium2 Bass kernel for nn_Attention_60189671686666 (gnn_message_passing).

Math per batch b (N=128 atoms, H=256 feats):
    P        = X @ W_pair                       (N, H)   [pre-projection trick]
    S_raw[i,j] = sum_k relu(P[i,k] + P[j,k]) * W_att[k]  (symmetric in i,j)
    S        = sigmoid(S_raw + b_att)
    out      = S @ X

Distribution: data-parallel over batch B=32 across 8 NeuronCores (4 each).
No collectives needed; params are replicated.

On-core layout: P is kept transposed (P_T: feature k on partitions, atom n on
free axis) so the per-column bias P[:, i] is a per-partition scalar for
tensor_scalar / activation. The relu'd pairwise tile (bf16) is consumed
immediately by a PE matmul against W_att, accumulating score columns in PSUM.
Produce work is split DVE:ACT ~66:34 (per-tile cost ~163ns vs ~293ns, both
dominated by the per-instruction SBUF-access bubble, which bounds this kernel:
8192 produce instructions fleet-wide is the floor for this ISA since the
per-partition bias pins each instruction to one 128x128 tile).
Measured ~137.5 us on hardware (DVE 83% busy), rel err 2.6e-3.
"""

import sys
import numpy as np

if "/opt/trn_rl_repo" not in sys.path:
    sys.path.insert(0, "/opt/trn_rl_repo")

B, N, H = 32, 128, 256
NCORES = 8
BL = B // NCORES  # batches per core

_CACHE = {}

# debug knobs (bisection): stages = 0 io-only, 1 +transpose/P, 2 +scores, 3 full
import os
DBG_STAGE = int(os.environ.get("K_DBG_STAGE", "3"))
DBG_NB = int(os.environ.get("K_DBG_NB", str(BL)))
DBG_NI = int(os.environ.get("K_DBG_NI", str(N)))


def _build_nc():
    import concourse.bass as bass
    import concourse.tile as tile
    from concourse import bacc, mybir

    f32 = mybir.dt.float32
    bf16 = mybir.dt.bfloat16
    RELU = mybir.ActivationFunctionType.Relu
    SIGMOID = mybir.ActivationFunctionType.Sigmoid
    ADD = mybir.AluOpType.add
    MAX = mybir.AluOpType.max

    nc = bacc.Bacc(None, target_bir_lowering=False, debug=False)
    lf = nc.declare_dram_parameter("local_feats", [BL, N, H], f32, isOutput=False)
    wp = nc.declare_dram_parameter("W_pair", [H, H], f32, isOutput=False)
    wa = nc.declare_dram_parameter("W_att", [H, 1], f32, isOutput=False)
    ba = nc.declare_dram_parameter("b_att", [1], f32, isOutput=False)
    out = nc.declare_dram_parameter("out", [BL, N, H], f32, isOutput=True)

    ident = nc.inline_tensor(np.eye(128, dtype=np.float32), name="ident")

    with tile.TileContext(nc) as tc:
        with (
            tc.tile_pool(name="const", bufs=1) as cpool,
            tc.tile_pool(name="work", bufs=3) as wpool,
            tc.tile_pool(name="atiles", bufs=16) as apool,
            tc.tile_pool(name="psum", bufs=2, space="PSUM") as ppool,
            tc.tile_pool(name="psumS", bufs=3, space="PSUM") as spool,
            tc.tile_pool(name="psumO", bufs=1, space="PSUM") as opool,
        ):
            # ---- constants (once per core) ----
            # batch-0 features first so the transpose/P pipeline starts early
            x0_sb = wpool.tile([128, H], f32, tag="x")
            nc.sync.dma_start(x0_sb[:], lf[0])

            id_sb = cpool.tile([128, 128], f32)
            nc.sync.dma_start(id_sb[:], ident[:])

            # W_pair rows h on partitions: wp_sb[:, ht*H + k] = W_pair[ht*128 + p, k]
            wp_sb = cpool.tile([128, 2 * H], f32)
            for ht in range(2):
                nc.sync.dma_start(wp_sb[:, ht * H:(ht + 1) * H],
                                  wp[ht * 128:(ht + 1) * 128, :])

            w_sb = cpool.tile([128, 2], f32)
            nc.sync.dma_start(w_sb[:].rearrange("p (t o) -> p t o", t=2),
                              wa.rearrange("(t p) o -> p t o", p=128))
            w_bf = cpool.tile([128, 2], bf16)
            nc.vector.tensor_copy(w_bf[:], w_sb[:])

            # broadcast b_att to all 128 partitions with a 0-stride DMA read
            bias_sb = cpool.tile([128, 1], f32)
            ba_bc = ba[None, :]
            ba_bc.ap[0] = [0, 128]
            nc.sync.dma_start(bias_sb[:], ba_bc)

            # ---- per-batch pipeline ----
            for ib in range(DBG_NB):
                if ib == 0:
                    x_sb = x0_sb
                else:
                    x_sb = wpool.tile([128, H], f32, tag="x")
                    nc.sync.dma_start(x_sb[:], lf[ib])

                if DBG_STAGE == 0:
                    o_sb = wpool.tile([128, H], f32, tag="osb")
                    nc.vector.tensor_copy(o_sb[:], x_sb[:])
                    nc.sync.dma_start(out[ib], o_sb[:])
                    continue

                # X_T: (h on partitions, n on free), 2 h-tiles via PE transpose
                xT_sb = wpool.tile([128, H], f32, tag="xT")
                for ht in range(2):
                    xT_ps = ppool.tile([128, 128], f32, tag="xTp")
                    nc.tensor.transpose(xT_ps[:], x_sb[:, ht * 128:(ht + 1) * 128],
                                        id_sb[:])
                    nc.scalar.copy(xT_sb[:, ht * 128:(ht + 1) * 128], xT_ps[:])

                # P_T[k, n] = sum_h W_pair[h, k] * X[n, h]
                pT_ps = ppool.tile([128, H], f32, tag="pT")
                for kt in range(2):
                    for ht in range(2):
                        nc.tensor.matmul(
                            pT_ps[:, kt * 128:(kt + 1) * 128],
                            lhsT=wp_sb[:, ht * H + kt * 128: ht * H + kt * 128 + 128],
                            rhs=xT_sb[:, ht * 128:(ht + 1) * 128],
                            start=(ht == 0), stop=(ht == 1))

                # per-kt copies so kt0 produce can start while kt1's P matmuls
                # are still in flight
                pT_bf = wpool.tile([128, H], bf16, tag="pTbf")
                pT_f = wpool.tile([128, H], f32, tag="pTf")
                for kt in range(2):
                    sl = slice(kt * 128, (kt + 1) * 128)
                    nc.scalar.copy(pT_bf[:, sl], pT_ps[:, sl])
                    nc.scalar.copy(pT_f[:, sl], pT_ps[:, sl])
                x_bf = wpool.tile([128, H], bf16, tag="xbf")
                nc.scalar.copy(x_bf[:], x_sb[:])

                if DBG_STAGE == 1:
                    o_sb = wpool.tile([128, H], f32, tag="osb")
                    nc.vector.tensor_copy(o_sb[:], pT_f[:])
                    nc.sync.dma_start(out[ib], o_sb[:])
                    continue

                # scores: S_ps[j, i] = sum_k relu(P[j,k] + P[i,k]) w[k]
                S_ps = spool.tile([128, 128], f32, tag="S")
                for i in range(DBG_NI):
                    for kt in range(2):
                        u = (2 * i + kt) % 34
                        a_t = apool.tile([128, 128], bf16, tag=f"a{kt}")
                        src = pT_bf[:, kt * 128:(kt + 1) * 128]
                        bias_ap = pT_f[:, kt * 128 + i: kt * 128 + i + 1]
                        if u < 11:
                            nc.scalar.activation(a_t[:], src, RELU,
                                                 bias=bias_ap, scale=1.0)
                        else:
                            nc.vector.tensor_scalar(a_t[:], src, bias_ap, 0.0,
                                                    ADD, MAX)
                        nc.tensor.matmul(S_ps[:, i:i + 1], lhsT=a_t[:],
                                         rhs=w_bf[:, kt:kt + 1],
                                         start=(kt == 0), stop=(kt == 1))

                s_sig = wpool.tile([128, 128], bf16, tag="ssig")
                nc.scalar.activation(s_sig[:, 0:DBG_NI], S_ps[:, 0:DBG_NI], SIGMOID,
                                     bias=bias_sb[:, 0:1], scale=1.0)

                if DBG_STAGE == 2:
                    o_sb = wpool.tile([128, H], f32, tag="osb")
                    nc.vector.tensor_copy(o_sb[:, 0:128], s_sig[:])
                    nc.vector.tensor_copy(o_sb[:, 128:256], s_sig[:])
                    nc.sync.dma_start(out[ib], o_sb[:])
                    continue

                # out[i, h] = sum_j S[i,j] X[j,h]; S symmetric so lhsT = S works
                o_ps = opool.tile([128, H], f32, tag="ops")
                nc.tensor.matmul(o_ps[:], lhsT=s_sig[:], rhs=x_bf[:],
                                 start=True, stop=True)
                o_sb = wpool.tile([128, H], f32, tag="osb")
                nc.scalar.copy(o_sb[:], o_ps[:])
                nc.sync.dma_start(out[ib], o_sb[:])

    nc.compile()
    return nc


def _get_nc():
    if "nc" not in _CACHE:
        _CACHE["nc"] = _build_nc()
    return _CACHE["nc"]


def run(inputs, trace=False, **kw):
    from concourse.bass_utils import run_bass_kernel_spmd

    nc = _get_nc()
    lf = np.ascontiguousarray(inputs["local_feats"], dtype=np.float32)
    wp = np.ascontiguousarray(inputs["W_pair"], dtype=np.float32)
    wa = np.ascontiguousarray(inputs["W_att"], dtype=np.float32)
    ba = np.ascontiguousarray(inputs["b_att"], dtype=np.float32)
    in_maps = [
        {"local_feats": lf[c * BL:(c + 1) * BL], "W_pair": wp, "W_att": wa,
         "b_att": ba}
        for c in range(NCORES)
    ]
    res = run_bass_kernel_spmd(nc, in_maps, core_ids=list(range(NCORES)),
                               trace=trace, **kw)
    outp = np.concatenate([res.results[c]["out"] for c in range(NCORES)], axis=0)
    return outp.astype(np.float32), res


def kernel(**inputs):
    outp, _ = run(inputs, trace=False)
    return outp
